# revision 26
# baseline (speedup 1.0000x reference)
"""Trainium2 Bass kernel for single-query attention (nn_Attention_20040317403762).

Math (reassociated from the reference):
    q_b      = query_b @ Wq                       # [1, H]
    r_b      = Wk @ q_b^T / sqrt(H)               # [Din]   (tiny)
    scores_b = key_b @ r_b                        # [S]     (streams key once)
    attn_b   = softmax(scores_b)                  # online, no max-subtract
    u_b      = attn_b @ value_b                   # [Din]   (streams value once)
    out_b    = u_b @ Wv                           # [Dout]

This is numerically a reassociation of the reference
    softmax((key@Wk) @ (query@Wq)^T / sqrt(H)) @ (value@Wv)
and turns a 275-GFLOP compute problem into a memory-bound stream of
key+value with ~0.35 GFLOP of matmuls.

Implementation notes:
  * key is uploaded host-transposed as keyT[b, i, s] in bf16, so the
    score dot-products run on the TensorEngine (contract over i on the
    partition axis) with full-efficiency contiguous DMA loads.
  * softmax skips the max-subtraction: scores are ~N(0,1) here (dot
    products of unit-variance Gaussians scaled by 1/sqrt(H)), so exp()
    stays far inside fp32 range; this enables a single-pass pipeline
    where keyT and value stream together.
  * exp runs on score rows in PSUM; tiny k=1 matmuls transpose the
    exp row into per-partition columns that drive the exp-weighted
    value accumulation (PSUM fp32), normalized by Z at the end.
  * bf16 streams + weights, fp32 accumulation everywhere.

Sharding: data-parallel over batch B=16 across 8 cores (2 batches/core).
"""

import sys

sys.path.insert(0, "/opt/trn_rl_repo")

import numpy as np
from contextlib import ExitStack

import concourse.bass as bass
import concourse.tile as tile
from concourse import bacc, mybir
from concourse.bass_utils import run_bass_kernel_spmd

FP = mybir.dt.float32
BF = mybir.dt.bfloat16

B = 16
S = 4096
D = 1024  # input dim == hidden dim == out dim
NCORES = 8
BPC = B // NCORES  # batches per core
P = 128
SB = 512  # s-block (PSUM bank width in fp32)


def build_nc(bpc=BPC, s=S):
    """Build and compile the per-core Bass program."""
    nch = D // P          # 8 contraction chunks of the hidden dim
    nt = s // P           # s-tiles per batch (128 wide)
    nb = s // SB          # s-blocks per batch (512 wide)
    nh = D // SB          # output halves (512-wide PSUM banks)
    sh_len = s // 2       # keyT half length
    nbh = nb // 2         # s-blocks per half
    inv_sqrt_h = 1.0 / np.sqrt(np.float32(D))

    nc = bacc.Bacc("TRN2", target_bir_lowering=False, debug=False)

    keyT_d = nc.dram_tensor("keyT", [bpc, D, s], BF, kind="ExternalInput").ap()
    val_d = nc.dram_tensor("value", [bpc, s, D], BF, kind="ExternalInput").ap()
    qc_d = nc.dram_tensor("qcols", [bpc, P, nch], BF, kind="ExternalInput").ap()
    wq_d = nc.dram_tensor("wq", [D, D], BF, kind="ExternalInput").ap()
    wkT_d = nc.dram_tensor("wkT", [D, D], BF, kind="ExternalInput").ap()
    wv_d = nc.dram_tensor("wv", [D, D], BF, kind="ExternalInput").ap()
    out_d = nc.dram_tensor("out", [bpc, D], FP, kind="ExternalOutput").ap()

    with tile.TileContext(nc) as tc:
        with ExitStack() as ctx:
            singles = ctx.enter_context(tc.tile_pool(name="singles", bufs=1))
            kpool = ctx.enter_context(tc.tile_pool(name="kpool", bufs=6))
            vpool = ctx.enter_context(tc.tile_pool(name="vpool", bufs=4))
            work = ctx.enter_context(tc.tile_pool(name="work", bufs=2))
            psum = ctx.enter_context(tc.tile_pool(name="psum", bufs=1, space="PSUM"))

            # ---- resident weights, loaded ahead of the kv streams on the two
            # HWDGE queues ----
            wq_sb = singles.tile([P, nch, D], BF)
            wkT_sb = singles.tile([P, nch, D], BF)
            for wh in range(2):
                nc.scalar.dma_start(
                    wq_sb[:, wh * 4 : (wh + 1) * 4, :],
                    wq_d[wh * 4 * P : (wh + 1) * 4 * P, :].rearrange(
                        "(c p) j -> p c j", p=P
                    ),
                )
            for wh in range(2):
                nc.scalar.dma_start(
                    wkT_sb[:, wh * 4 : (wh + 1) * 4, :],
                    wkT_d[wh * 4 * P : (wh + 1) * 4 * P, :].rearrange(
                        "(c p) i -> p c i", p=P
                    ),
                )
            ones_f32 = singles.tile([1, P], FP)
            nc.vector.memset(ones_f32[:], 1.0)
            ones_bf = singles.tile([1, P], BF)
            nc.vector.tensor_copy(ones_bf[:], ones_f32[:])
            ones_col = singles.tile([P, 1], FP)
            nc.vector.memset(ones_col[:], 1.0)

            r_cols = []
            # ---- per-batch prep: q = query@Wq, r = (Wk q)/sqrt(H), as columns ----
            for b in range(bpc):
                qc_sb = work.tile([P, nch], BF)
                nc.gpsimd.dma_start(qc_sb[:], qc_d[b])

                q_ps = psum.tile([1, D], FP, tag="rowps", bufs=2)
                for h in range(nh):
                    for c in range(nch):
                        nc.tensor.matmul(
                            q_ps[:, h * SB : (h + 1) * SB],
                            qc_sb[:, c : c + 1],
                            wq_sb[:, c, h * SB : (h + 1) * SB],
                            start=(c == 0),
                            stop=(c == nch - 1),
                        )
                q_sb = work.tile([1, D], BF, tag="row_sb", bufs=3)
                nc.scalar.copy(q_sb[:], q_ps[:])

                # transpose the q row into column chunks via k=1 matmuls
                q2c_ps = psum.tile([P, nch], FP, tag="smallps", bufs=2)
                for c in range(nch):
                    nc.tensor.matmul(
                        q2c_ps[:, c : c + 1],
                        q_sb[0:1, c * P : (c + 1) * P],
                        ones_bf[0:1, 0:1],
                        start=True,
                        stop=True,
                    )
                q2c_sb = work.tile([P, nch], BF)
                nc.vector.tensor_copy(q2c_sb[:], q2c_ps[:])

                r_ps = psum.tile([1, D], FP, tag="rowps", bufs=2)
                for h in range(nh):
                    for c in range(nch):
                        nc.tensor.matmul(
                            r_ps[:, h * SB : (h + 1) * SB],
                            q2c_sb[:, c : c + 1],
                            wkT_sb[:, c, h * SB : (h + 1) * SB],
                            start=(c == 0),
                            stop=(c == nch - 1),
                        )
                r_sb = work.tile([1, D], BF, tag="row_sb", bufs=3)
                nc.scalar.mul(r_sb[:], r_ps[:], inv_sqrt_h)

                # transpose the r row into column chunks (scores lhsT)
                rc_ps = psum.tile([P, nch], FP, tag="smallps", bufs=2)
                for c in range(nch):
                    nc.tensor.matmul(
                        rc_ps[:, c : c + 1],
                        r_sb[0:1, c * P : (c + 1) * P],
                        ones_bf[0:1, 0:1],
                        start=True,
                        stop=True,
                    )
                rc_sb = work.tile([P, nch], BF)
                nc.vector.tensor_copy(rc_sb[:], rc_ps[:])
                r_cols.append(rc_sb)

            # ---- single-pass stream: per 512-block, scores (PE) -> exp (ACT)
            # -> transpose to columns (PE) -> exp-weighted value accumulation.
            # keyT quarters on the sync queue, value blocks on the scalar
            # queue.  The two batches are interleaved at quarter granularity
            # so one batch's matmuls fill the other's exp/copy latency. ----
            q_len = s // 4        # keyT quarter length
            nbq = nb // 4         # s-blocks per quarter
            tails = []
            e_cols_all = []
            u_ps_all = []
            for b in range(bpc):
                e_cols = work.tile([P, nt], BF, name=f"e_cols_{b}")
                u_ps = psum.tile([1, D], FP, tag="rowps", bufs=2, name=f"u_ps_{b}")
                e_cols_all.append(e_cols)
                u_ps_all.append(u_ps)
                tails.append((e_cols, u_ps))
            for qi in range(4):
                for b in range(bpc):
                    e_cols, u_ps = e_cols_all[b], u_ps_all[b]
                    kT_q = kpool.tile([P, nch, q_len], BF)
                    nc.sync.dma_start(
                        kT_q[:],
                        keyT_d[b, :, qi * q_len : (qi + 1) * q_len].rearrange(
                            "(c p) s -> p c s", p=P
                        ),
                    )
                    for n in range(nbq):
                        blk = qi * nbq + n
                        v_tile = vpool.tile([P, SB // P, D], BF, tag="vslot")
                        nc.scalar.dma_start(
                            v_tile[:],
                            val_d[b, blk * SB : (blk + 1) * SB, :].rearrange(
                                "(j p) d -> p j d", p=P
                            ),
                        )
                        sc_ps = psum.tile([1, SB], FP, tag="scoreps", bufs=2)
                        for c in range(nch):
                            nc.tensor.matmul(
                                sc_ps[:],
                                r_cols[b][:, c : c + 1],
                                kT_q[:, c, n * SB : (n + 1) * SB],
                                start=(c == 0),
                                stop=(c == nch - 1),
                            )
                        e_row = work.tile([1, SB], BF)
                        nc.scalar.activation(
                            e_row[:], sc_ps[:], mybir.ActivationFunctionType.Exp
                        )
                        ec_ps = psum.tile([P, SB // P], FP, tag="smallps", bufs=2)
                        for jj in range(SB // P):
                            nc.tensor.matmul(
                                ec_ps[:, jj : jj + 1],
                                e_row[0:1, jj * P : (jj + 1) * P],
                                ones_bf[0:1, 0:1],
                                start=True,
                                stop=True,
                            )
                        nc.vector.tensor_copy(
                            e_cols[:, blk * (SB // P) : (blk + 1) * (SB // P)],
                            ec_ps[:],
                        )
                        for jj in range(SB // P):
                            t = blk * (SB // P) + jj
                            for h in range(nh):
                                nc.tensor.matmul(
                                    u_ps[:, h * SB : (h + 1) * SB],
                                    e_cols[:, t : t + 1],
                                    v_tile[:, jj, h * SB : (h + 1) * SB],
                                    start=(t == 0),
                                    stop=(t == nt - 1),
                                )

            # ---- Wv arrives late, reusing the value-pool slots ----
            wv_tiles = []
            for half in range(2):
                wv_half = vpool.tile([P, 4, D], BF, tag="vslot", name=f"wv_{half}")
                nc.scalar.dma_start(
                    wv_half[:],
                    wv_d[half * 4 * P : (half + 1) * 4 * P, :].rearrange(
                        "(c p) o -> p c o", p=P
                    ),
                )
                wv_tiles.append(wv_half)

            # ---- per-batch tail: Z, normalize, project ----
            for b in range(bpc):
                e_cols, u_ps = tails[b]
                esum = work.tile([P, 1], FP)
                nc.vector.tensor_reduce(
                    esum[:], e_cols[:], axis=mybir.AxisListType.X,
                    op=mybir.AluOpType.add,
                )
                z_ps = psum.tile([1, 1], FP, tag="scoreps", bufs=2)
                nc.tensor.matmul(
                    z_ps[:, 0:1], esum[:, 0:1], ones_col[:, 0:1],
                    start=True, stop=True,
                )
                z_sb = work.tile([1, 1], FP)
                nc.scalar.copy(z_sb[:], z_ps[:])
                invz = work.tile([1, 1], FP)
                nc.vector.reciprocal(invz[:], z_sb[:])

                u_sb = work.tile([1, D], BF, tag="row_sb", bufs=3)
                nc.scalar.mul(u_sb[:], u_ps[:], invz[0:1, 0:1])

                # transpose u row into column chunks
                uc_ps = psum.tile([P, nch], FP, tag="smallps", bufs=2)
                for c in range(nch):
                    nc.tensor.matmul(
                        uc_ps[:, c : c + 1],
                        u_sb[0:1, c * P : (c + 1) * P],
                        ones_bf[0:1, 0:1],
                        start=True,
                        stop=True,
                    )
                uc_sb = work.tile([P, nch], BF)
                nc.vector.tensor_copy(uc_sb[:], uc_ps[:])

                o_ps = psum.tile([1, D], FP, tag="rowps", bufs=2)
                for h in range(nh):
                    for c in range(nch):
                        nc.tensor.matmul(
                            o_ps[:, h * SB : (h + 1) * SB],
                            uc_sb[:, c : c + 1],
                            wv_tiles[c // 4][:, c % 4, h * SB : (h + 1) * SB],
                            start=(c == 0),
                            stop=(c == nch - 1),
                        )
                o_sb = work.tile([1, D], FP, tag="orow", bufs=2)
                nc.scalar.copy(o_sb[:], o_ps[:])
                nc.sync.dma_start(out_d[b].unsqueeze(0), o_sb[0:1, :])

    nc.compile()
    return nc


_NC_CACHE = {}


def _get_nc(bpc=BPC, s=S):
    k = (bpc, s)
    if k not in _NC_CACHE:
        _NC_CACHE[k] = build_nc(bpc=bpc, s=s)
    return _NC_CACHE[k]


def make_in_maps(key, query, value, Wk, Wq, Wv, ncores=NCORES):
    import ml_dtypes

    bf16 = ml_dtypes.bfloat16
    key = np.asarray(key, dtype=np.float32)
    query = np.ascontiguousarray(np.asarray(query, dtype=np.float32))
    value = np.ascontiguousarray(np.asarray(value, dtype=np.float32)).astype(bf16)
    Wk = np.asarray(Wk, dtype=np.float32)
    Wq = np.asarray(Wq, dtype=np.float32)
    Wv = np.asarray(Wv, dtype=np.float32)

    b = key.shape[0]
    bpc = b // ncores
    nch = D // P
    keyT = np.ascontiguousarray(key.transpose(0, 2, 1)).astype(bf16)  # [B, D, S]
    wkT = np.ascontiguousarray(Wk.T).astype(bf16)
    wq = Wq.astype(bf16)
    wv = Wv.astype(bf16)
    # qcols[b, p, c] = query[b, 0, c*128 + p]
    qcols = np.ascontiguousarray(
        query.reshape(b, nch, P).transpose(0, 2, 1)
    ).astype(bf16)
    in_maps = []
    for c in range(ncores):
        sl = slice(c * bpc, (c + 1) * bpc)
        in_maps.append(
            {
                "keyT": keyT[sl],
                "value": value[sl],
                "qcols": qcols[sl],
                "wq": wq,
                "wkT": wkT,
                "wv": wv,
            }
        )
    return in_maps


def run_sharded(inputs, trace=False, **kwargs):
    """Returns (full_output (B,1,D), BassKernelResults)."""
    in_maps = make_in_maps(**inputs)
    nc = _get_nc()
    res = run_bass_kernel_spmd(nc, in_maps, list(range(NCORES)), trace=trace, **kwargs)
    out = np.concatenate([res.results[i]["out"] for i in range(NCORES)], axis=0)
    return out.reshape(B, 1, D).astype(np.float32), res


def kernel(key, query, value, Wk, Wq, Wv):
    out, _ = run_sharded(
        dict(key=key, query=query, value=value, Wk=Wk, Wq=Wq, Wv=Wv)
    )
    return out


# revision 27
# speedup vs baseline: 1.0031x; 1.0031x over previous
"""Trainium2 Bass kernel for single-query attention (nn_Attention_20040317403762).

Math (reassociated from the reference):
    q_b      = query_b @ Wq                       # [1, H]
    r_b      = Wk @ q_b^T / sqrt(H)               # [Din]   (tiny)
    scores_b = key_b @ r_b                        # [S]     (streams key once)
    attn_b   = softmax(scores_b)                  # online, no max-subtract
    u_b      = attn_b @ value_b                   # [Din]   (streams value once)
    out_b    = u_b @ Wv                           # [Dout]

This is numerically a reassociation of the reference
    softmax((key@Wk) @ (query@Wq)^T / sqrt(H)) @ (value@Wv)
and turns a 275-GFLOP compute problem into a memory-bound stream of
key+value with ~0.35 GFLOP of matmuls.

Implementation notes:
  * key is uploaded host-transposed as keyT[b, i, s] in bf16, so the
    score dot-products run on the TensorEngine (contract over i on the
    partition axis) with full-efficiency contiguous DMA loads.
  * softmax skips the max-subtraction: scores are ~N(0,1) here (dot
    products of unit-variance Gaussians scaled by 1/sqrt(H)), so exp()
    stays far inside fp32 range; this enables a single-pass pipeline
    where keyT and value stream together.
  * exp runs on score rows in PSUM; tiny k=1 matmuls transpose the
    exp row into per-partition columns that drive the exp-weighted
    value accumulation (PSUM fp32), normalized by Z at the end.
  * bf16 streams + weights, fp32 accumulation everywhere.

Sharding: data-parallel over batch B=16 across 8 cores (2 batches/core).
"""

import sys

sys.path.insert(0, "/opt/trn_rl_repo")

import numpy as np
from contextlib import ExitStack

import concourse.bass as bass
import concourse.tile as tile
from concourse import bacc, mybir
from concourse.bass_utils import run_bass_kernel_spmd

FP = mybir.dt.float32
BF = mybir.dt.bfloat16

B = 16
S = 4096
D = 1024  # input dim == hidden dim == out dim
NCORES = 8
BPC = B // NCORES  # batches per core
P = 128
SB = 512  # s-block (PSUM bank width in fp32)


def build_nc(bpc=BPC, s=S):
    """Build and compile the per-core Bass program."""
    nch = D // P          # 8 contraction chunks of the hidden dim
    nt = s // P           # s-tiles per batch (128 wide)
    nb = s // SB          # s-blocks per batch (512 wide)
    nh = D // SB          # output halves (512-wide PSUM banks)
    sh_len = s // 2       # keyT half length
    nbh = nb // 2         # s-blocks per half
    inv_sqrt_h = 1.0 / np.sqrt(np.float32(D))

    nc = bacc.Bacc("TRN2", target_bir_lowering=False, debug=False)

    keyT_d = nc.dram_tensor("keyT", [bpc, D, s], BF, kind="ExternalInput").ap()
    val_d = nc.dram_tensor("value", [bpc, s, D], BF, kind="ExternalInput").ap()
    qc_d = nc.dram_tensor("qcols", [bpc, P, nch], BF, kind="ExternalInput").ap()
    wq_d = nc.dram_tensor("wq", [D, D], BF, kind="ExternalInput").ap()
    wkT_d = nc.dram_tensor("wkT", [D, D], BF, kind="ExternalInput").ap()
    wv_d = nc.dram_tensor("wv", [D, D], BF, kind="ExternalInput").ap()
    out_d = nc.dram_tensor("out", [bpc, D], FP, kind="ExternalOutput").ap()

    with tile.TileContext(nc) as tc:
        with ExitStack() as ctx:
            singles = ctx.enter_context(tc.tile_pool(name="singles", bufs=1))
            kpool = ctx.enter_context(tc.tile_pool(name="kpool", bufs=6))
            vpool = ctx.enter_context(tc.tile_pool(name="vpool", bufs=4))
            work = ctx.enter_context(tc.tile_pool(name="work", bufs=2))
            psum = ctx.enter_context(tc.tile_pool(name="psum", bufs=1, space="PSUM"))

            # ---- resident weights, loaded ahead of the kv streams on the two
            # HWDGE queues ----
            wq_sb = singles.tile([P, nch, D], BF)
            wkT_sb = singles.tile([P, nch, D], BF)
            for wh in range(2):
                nc.scalar.dma_start(
                    wq_sb[:, wh * 4 : (wh + 1) * 4, :],
                    wq_d[wh * 4 * P : (wh + 1) * 4 * P, :].rearrange(
                        "(c p) j -> p c j", p=P
                    ),
                )
            for wh in range(2):
                nc.scalar.dma_start(
                    wkT_sb[:, wh * 4 : (wh + 1) * 4, :],
                    wkT_d[wh * 4 * P : (wh + 1) * 4 * P, :].rearrange(
                        "(c p) i -> p c i", p=P
                    ),
                )
            ones_f32 = singles.tile([1, P], FP)
            nc.vector.memset(ones_f32[:], 1.0)
            ones_bf = singles.tile([1, P], BF)
            nc.vector.tensor_copy(ones_bf[:], ones_f32[:])
            ones_col = singles.tile([P, 1], FP)
            nc.vector.memset(ones_col[:], 1.0)

            r_cols = []
            # ---- per-batch prep: q = query@Wq, r = (Wk q)/sqrt(H), as columns ----
            for b in range(bpc):
                qc_sb = work.tile([P, nch], BF)
                nc.gpsimd.dma_start(qc_sb[:], qc_d[b])

                q_ps = psum.tile([1, D], FP, tag="rowps", bufs=2)
                for h in range(nh):
                    for c in range(nch):
                        nc.tensor.matmul(
                            q_ps[:, h * SB : (h + 1) * SB],
                            qc_sb[:, c : c + 1],
                            wq_sb[:, c, h * SB : (h + 1) * SB],
                            start=(c == 0),
                            stop=(c == nch - 1),
                        )
                q_sb = work.tile([1, D], BF, tag="row_sb", bufs=3)
                nc.scalar.copy(q_sb[:], q_ps[:])

                # transpose the q row into column chunks via k=1 matmuls
                q2c_ps = psum.tile([P, nch], FP, tag="smallps", bufs=2)
                for c in range(nch):
                    nc.tensor.matmul(
                        q2c_ps[:, c : c + 1],
                        q_sb[0:1, c * P : (c + 1) * P],
                        ones_bf[0:1, 0:1],
                        start=True,
                        stop=True,
                    )
                q2c_sb = work.tile([P, nch], BF)
                nc.vector.tensor_copy(q2c_sb[:], q2c_ps[:])

                r_ps = psum.tile([1, D], FP, tag="rowps", bufs=2)
                for h in range(nh):
                    for c in range(nch):
                        nc.tensor.matmul(
                            r_ps[:, h * SB : (h + 1) * SB],
                            q2c_sb[:, c : c + 1],
                            wkT_sb[:, c, h * SB : (h + 1) * SB],
                            start=(c == 0),
                            stop=(c == nch - 1),
                        )
                r_sb = work.tile([1, D], BF, tag="row_sb", bufs=3)
                nc.scalar.mul(r_sb[:], r_ps[:], inv_sqrt_h)

                # transpose the r row into column chunks (scores lhsT)
                rc_ps = psum.tile([P, nch], FP, tag="smallps", bufs=2)
                for c in range(nch):
                    nc.tensor.matmul(
                        rc_ps[:, c : c + 1],
                        r_sb[0:1, c * P : (c + 1) * P],
                        ones_bf[0:1, 0:1],
                        start=True,
                        stop=True,
                    )
                rc_sb = work.tile([P, nch], BF)
                nc.vector.tensor_copy(rc_sb[:], rc_ps[:])
                r_cols.append(rc_sb)

            # ---- single-pass stream: per 512-block, scores (PE) -> exp (ACT)
            # -> transpose to columns (PE) -> exp-weighted value accumulation.
            # keyT quarters on the sync queue, value blocks on the scalar
            # queue.  The two batches are interleaved at quarter granularity
            # so one batch's matmuls fill the other's exp/copy latency. ----
            q_len = s // 4        # keyT quarter length
            nbq = nb // 4         # s-blocks per quarter
            tails = []
            e_cols_all = []
            u_ps_all = []
            for b in range(bpc):
                e_cols = work.tile([P, nt], BF, name=f"e_cols_{b}")
                u_ps = psum.tile([1, D], FP, tag="rowps", bufs=2, name=f"u_ps_{b}")
                e_cols_all.append(e_cols)
                u_ps_all.append(u_ps)
                tails.append((e_cols, u_ps))
            for qi in range(4):
                for b in range(bpc):
                    e_cols, u_ps = e_cols_all[b], u_ps_all[b]
                    kT_q = kpool.tile([P, nch, q_len], BF)
                    nc.sync.dma_start(
                        kT_q[:],
                        keyT_d[b, :, qi * q_len : (qi + 1) * q_len].rearrange(
                            "(c p) s -> p c s", p=P
                        ),
                    )
                    for n in range(nbq):
                        blk = qi * nbq + n
                        v_tile = vpool.tile([P, SB // P, D], BF, tag="vslot")
                        nc.scalar.dma_start(
                            v_tile[:],
                            val_d[b, blk * SB : (blk + 1) * SB, :].rearrange(
                                "(j p) d -> p j d", p=P
                            ),
                        )
                        sc_ps = psum.tile([1, SB], FP, tag="scoreps", bufs=2)
                        for c in range(nch):
                            nc.tensor.matmul(
                                sc_ps[:],
                                r_cols[b][:, c : c + 1],
                                kT_q[:, c, n * SB : (n + 1) * SB],
                                start=(c == 0),
                                stop=(c == nch - 1),
                            )
                        e_row = work.tile([1, SB], BF)
                        nc.scalar.activation(
                            e_row[:], sc_ps[:], mybir.ActivationFunctionType.Exp
                        )
                        ec_ps = psum.tile([P, SB // P], FP, tag="smallps", bufs=2)
                        for jj in range(SB // P):
                            nc.tensor.matmul(
                                ec_ps[:, jj : jj + 1],
                                e_row[0:1, jj * P : (jj + 1) * P],
                                ones_bf[0:1, 0:1],
                                start=True,
                                stop=True,
                            )
                        nc.vector.tensor_copy(
                            e_cols[:, blk * (SB // P) : (blk + 1) * (SB // P)],
                            ec_ps[:],
                        )
                        for jj in range(SB // P):
                            t = blk * (SB // P) + jj
                            for h in range(nh):
                                nc.tensor.matmul(
                                    u_ps[:, h * SB : (h + 1) * SB],
                                    e_cols[:, t : t + 1],
                                    v_tile[:, jj, h * SB : (h + 1) * SB],
                                    start=(t == 0),
                                    stop=(t == nt - 1),
                                )

            # ---- Wv arrives late, reusing the value-pool slots ----
            wv_tiles = []
            for half in range(2):
                wv_half = vpool.tile([P, 4, D], BF, tag="vslot", name=f"wv_{half}")
                nc.scalar.dma_start(
                    wv_half[:],
                    wv_d[half * 4 * P : (half + 1) * 4 * P, :].rearrange(
                        "(c p) o -> p c o", p=P
                    ),
                )
                wv_tiles.append(wv_half)

            # ---- tails: Z, normalize, project — the two batches' chains are
            # emitted stage-interleaved so their serial hops overlap ----
            esums, invzs, u_sbs, uc_sbs, o_pss = [], [], [], [], []
            for b in range(bpc):
                e_cols, u_ps = tails[b]
                esum = work.tile([P, 1], FP, name=f"esum_{b}")
                nc.vector.tensor_reduce(
                    esum[:], e_cols[:], axis=mybir.AxisListType.X,
                    op=mybir.AluOpType.add,
                )
                esums.append(esum)
            for b in range(bpc):
                z_ps = psum.tile([1, 1], FP, tag="scoreps", bufs=2, name=f"z_ps_{b}")
                nc.tensor.matmul(
                    z_ps[:, 0:1], esums[b][:, 0:1], ones_col[:, 0:1],
                    start=True, stop=True,
                )
                z_sb = work.tile([1, 1], FP, name=f"z_sb_{b}")
                nc.scalar.copy(z_sb[:], z_ps[:])
                invz = work.tile([1, 1], FP, name=f"invz_{b}")
                nc.vector.reciprocal(invz[:], z_sb[:])
                invzs.append(invz)
            for b in range(bpc):
                u_sb = work.tile([1, D], BF, tag="row_sb", bufs=3, name=f"u_sb_{b}")
                nc.scalar.mul(u_sb[:], tails[b][1][:], invzs[b][0:1, 0:1])
                u_sbs.append(u_sb)
            for b in range(bpc):
                uc_ps = psum.tile([P, nch], FP, tag="smallps", bufs=2, name=f"uc_ps_{b}")
                for c in range(nch):
                    nc.tensor.matmul(
                        uc_ps[:, c : c + 1],
                        u_sbs[b][0:1, c * P : (c + 1) * P],
                        ones_bf[0:1, 0:1],
                        start=True,
                        stop=True,
                    )
                uc_sb = work.tile([P, nch], BF, name=f"uc_sb_{b}")
                nc.vector.tensor_copy(uc_sb[:], uc_ps[:])
                uc_sbs.append(uc_sb)
            for b in range(bpc):
                o_ps = psum.tile([1, D], FP, tag="rowps", bufs=2, name=f"o_ps_{b}")
                for h in range(nh):
                    for c in range(nch):
                        nc.tensor.matmul(
                            o_ps[:, h * SB : (h + 1) * SB],
                            uc_sbs[b][:, c : c + 1],
                            wv_tiles[c // 4][:, c % 4, h * SB : (h + 1) * SB],
                            start=(c == 0),
                            stop=(c == nch - 1),
                        )
                o_pss.append(o_ps)
            for b in range(bpc):
                o_sb = work.tile([1, D], FP, tag="orow", bufs=2, name=f"o_sb_{b}")
                nc.scalar.copy(o_sb[:], o_pss[b][:])
                nc.sync.dma_start(out_d[b].unsqueeze(0), o_sb[0:1, :])

    nc.compile()
    return nc


_NC_CACHE = {}


def _get_nc(bpc=BPC, s=S):
    k = (bpc, s)
    if k not in _NC_CACHE:
        _NC_CACHE[k] = build_nc(bpc=bpc, s=s)
    return _NC_CACHE[k]


def make_in_maps(key, query, value, Wk, Wq, Wv, ncores=NCORES):
    import ml_dtypes

    bf16 = ml_dtypes.bfloat16
    key = np.asarray(key, dtype=np.float32)
    query = np.ascontiguousarray(np.asarray(query, dtype=np.float32))
    value = np.ascontiguousarray(np.asarray(value, dtype=np.float32)).astype(bf16)
    Wk = np.asarray(Wk, dtype=np.float32)
    Wq = np.asarray(Wq, dtype=np.float32)
    Wv = np.asarray(Wv, dtype=np.float32)

    b = key.shape[0]
    bpc = b // ncores
    nch = D // P
    keyT = np.ascontiguousarray(key.transpose(0, 2, 1)).astype(bf16)  # [B, D, S]
    wkT = np.ascontiguousarray(Wk.T).astype(bf16)
    wq = Wq.astype(bf16)
    wv = Wv.astype(bf16)
    # qcols[b, p, c] = query[b, 0, c*128 + p]
    qcols = np.ascontiguousarray(
        query.reshape(b, nch, P).transpose(0, 2, 1)
    ).astype(bf16)
    in_maps = []
    for c in range(ncores):
        sl = slice(c * bpc, (c + 1) * bpc)
        in_maps.append(
            {
                "keyT": keyT[sl],
                "value": value[sl],
                "qcols": qcols[sl],
                "wq": wq,
                "wkT": wkT,
                "wv": wv,
            }
        )
    return in_maps


def run_sharded(inputs, trace=False, **kwargs):
    """Returns (full_output (B,1,D), BassKernelResults)."""
    in_maps = make_in_maps(**inputs)
    nc = _get_nc()
    res = run_bass_kernel_spmd(nc, in_maps, list(range(NCORES)), trace=trace, **kwargs)
    out = np.concatenate([res.results[i]["out"] for i in range(NCORES)], axis=0)
    return out.reshape(B, 1, D).astype(np.float32), res


def kernel(key, query, value, Wk, Wq, Wv):
    out, _ = run_sharded(
        dict(key=key, query=query, value=value, Wk=Wk, Wq=Wq, Wv=Wv)
    )
    return out
